# revision 3
# baseline (speedup 1.0000x reference)
"""Trainium2 Bass kernel for ColumnConsistencyLoss (segment_reduce).

Problem: B=16, T=8192, C=128.
  probs = softmax(logits, -1)           # (N, C), N = B*T = 131072
  per column-id c (segment): n_c = #valid tokens, S_c = sum w*p, Q_c = sum w*p^2
  col_var_c = (sum_j Q_cj - sum_j S_cj^2 / n_safe_c) / (n_safe_c * C)
  loss = mean over columns with n_c > 1 of col_var_c

Sharding: data-parallel over tokens — each of the 8 cores processes
N/8 = 16384 tokens and produces partial per-segment accumulators
S (C x C) and Q (C x C).  The cross-core reduction of these tiny
accumulators plus the final scalar math happens on the host (exact
counts n_c are also computed on host via bincount).

Device kernel per core (single streaming sweep, tokens on partitions):
  - DMA logits in 4 chunks of (128p, 32j, 128c)   [token t = p*128 + j]
  - ScalarE: E = exp(L) -> bf16 (big-FD chunk)
  - DVE:     d = reduce_sum_c(E); r = 1/d; wr = r * w
  - DVE:     rhs1[:,j,0,:] = E[:,j,:] * wr[:,j]    (per-tile tensor_scalar)
  - ScalarE: rhs[:,:,1,:] = Square(rhs[:,:,0,:])   (big-FD chunk)
  - DVE:     M = (iota == seg_j)                   (one-hot, bf16)
  - PE:      psum[(c),(s,j')] += M^T @ rhs[:,j,:,:]  (F=256, fp32 accum)
The matmul contracts over the 128 partitions (tokens), so
  psum[c,0,:] = sum_t 1[seg=c] * w*E/d        = S_c
  psum[c,1,:] = sum_t 1[seg=c] * (w*E/d)^2    = Q_c   (w^2 = w).
"""

import numpy as np
import ml_dtypes

NCORES = 8
P = 128           # partitions
C = 128           # columns / segments
B, T = 16, 8192
N_TOK = B * T
TOK_PER_CORE = N_TOK // NCORES   # 16384
J_FULL = TOK_PER_CORE // P       # 128 free-columns (token tiles) per core
CHUNK_J = 32                     # token tiles per DMA/compute chunk

TRACE = False          # set True (e.g. from test.py) to capture NTFF profile
TRACE_TMPDIR = None    # where trace/NEFF artifacts land when TRACE is set
LAST_RESULT = None     # BassKernelResults of the last run (for profiling)

_NC_CACHE = {}


def build_nc(j_full=J_FULL, chunk_j=CHUNK_J):
    """Build + compile the Bass program (SPMD; same NEFF on all cores)."""
    from concourse import bacc, mybir
    import concourse.tile as tile

    f32 = mybir.dt.float32
    bf16 = mybir.dt.bfloat16
    Exp = mybir.ActivationFunctionType.Exp
    Square = mybir.ActivationFunctionType.Square
    Alu = mybir.AluOpType

    tok = j_full * P
    nchunk = j_full // chunk_j

    nc = bacc.Bacc("TRN2", target_bir_lowering=False, debug=False,
                   enable_asserts=False)

    lg_d = nc.dram_tensor("logits", [tok, C], f32, kind="ExternalInput")
    seg_d = nc.dram_tensor("segf", [P, j_full], f32, kind="ExternalInput")
    w_d = nc.dram_tensor("wf", [P, j_full], f32, kind="ExternalInput")
    iota_d = nc.dram_tensor("iotab", [P, C], bf16, kind="ExternalInput")
    sq_d = nc.dram_tensor("sq_out", [C, 2, C], f32, kind="ExternalOutput")

    with tile.TileContext(nc) as tc:
        with (
            tc.tile_pool(name="const", bufs=1) as constp,
            tc.tile_pool(name="big", bufs=3) as bigp,
            tc.tile_pool(name="small", bufs=3) as smallp,
            tc.tile_pool(name="mpool", bufs=6) as mpool,
            tc.tile_pool(name="psum", bufs=1, space="PSUM") as psump,
        ):
            iota_t = constp.tile([P, C], bf16)
            seg_t = constp.tile([P, j_full], f32)
            w_t = constp.tile([P, j_full], f32)
            nc.sync.dma_start(iota_t[:], iota_d[:])
            nc.sync.dma_start(seg_t[:], seg_d[:])
            nc.sync.dma_start(w_t[:], w_d[:])

            psum_sq = psump.tile([C, 2, C], f32)

            # DRAM view: (p, j, c) with token t = p*j_full + j
            lg_ap = lg_d[:].rearrange("(p j) c -> p j c", j=j_full)

            for k in range(nchunk):
                js = k * chunk_j
                L = bigp.tile([P, chunk_j, C], f32, tag="L")
                nc.sync.dma_start(L[:], lg_ap[:, js:js + chunk_j, :])

                E = bigp.tile([P, chunk_j, C], bf16, tag="E")
                nc.scalar.activation(E[:], L[:], Exp)

                d = smallp.tile([P, chunk_j], f32, tag="d")
                nc.vector.tensor_reduce(d[:], E[:], axis=mybir.AxisListType.X,
                                        op=Alu.add)
                r = smallp.tile([P, chunk_j], f32, tag="r")
                nc.vector.reciprocal(r[:], d[:])
                wr = smallp.tile([P, chunk_j], f32, tag="wr")
                nc.vector.tensor_tensor(wr[:], r[:], w_t[:, js:js + chunk_j],
                                        op=Alu.mult)

                rhs = bigp.tile([P, chunk_j, 2, C], bf16, tag="rhs")
                for jj in range(chunk_j):
                    nc.vector.tensor_scalar(
                        rhs[:, jj, 0, :], E[:, jj, :],
                        wr[:, jj:jj + 1], None, op0=Alu.mult)
                # squared half for the whole chunk in one ACT instruction
                nc.scalar.activation(rhs[:, :, 1, :], rhs[:, :, 0, :], Square)

                for jj in range(chunk_j):
                    j = js + jj
                    m_t = mpool.tile([P, C], bf16, tag="M")
                    nc.vector.tensor_scalar(
                        m_t[:], iota_t[:], seg_t[:, j:j + 1], None,
                        op0=Alu.is_equal)
                    nc.tensor.matmul(
                        psum_sq[:], m_t[:], rhs[:, jj, :, :],
                        start=(j == 0), stop=(j == j_full - 1))

            out_t = constp.tile([C, 2, C], f32)
            nc.scalar.copy(out_t[:], psum_sq[:])
            nc.sync.dma_start(sq_d[:], out_t[:])

    nc.compile()
    return nc


def _get_nc():
    key = (J_FULL, CHUNK_J)
    if key not in _NC_CACHE:
        _NC_CACHE[key] = build_nc(*key)
    return _NC_CACHE[key]


def kernel(column_logits, column_assignments, valid_mask):
    global LAST_RESULT
    from concourse.bass_utils import run_bass_kernel_spmd

    logits = np.asarray(column_logits, dtype=np.float32).reshape(N_TOK, C)
    seg = np.asarray(column_assignments).reshape(N_TOK).astype(np.int64)
    w = np.asarray(valid_mask).reshape(N_TOK).astype(bool)

    iotab = np.ascontiguousarray(
        np.broadcast_to(np.arange(C, dtype=np.float32), (P, C))
    ).astype(ml_dtypes.bfloat16)

    in_maps = []
    for i in range(NCORES):
        sl = slice(i * TOK_PER_CORE, (i + 1) * TOK_PER_CORE)
        in_maps.append({
            "logits": np.ascontiguousarray(logits[sl]),
            "segf": np.ascontiguousarray(
                seg[sl].reshape(P, J_FULL).astype(np.float32)),
            "wf": np.ascontiguousarray(
                w[sl].reshape(P, J_FULL).astype(np.float32)),
            "iotab": iotab,
        })

    nc = _get_nc()
    res = run_bass_kernel_spmd(nc, in_maps, list(range(NCORES)), trace=TRACE,
                               tmpdir=TRACE_TMPDIR)
    LAST_RESULT = res

    SQ = np.zeros((C, 2, C), np.float64)
    for rm in res.results:
        SQ += np.asarray(rm["sq_out"], dtype=np.float64)
    S = SQ[:, 0, :]
    Q = SQ[:, 1, :]

    n = np.bincount(seg[w], minlength=C).astype(np.float64)
    n_safe = np.maximum(n, 1.0)
    ssd_sum = Q.sum(axis=1) - (S * S).sum(axis=1) / n_safe
    col_var = ssd_sum / (n_safe * C)
    has_multi = n > 1.0
    count = has_multi.sum()
    total = np.where(has_multi, col_var, 0.0).sum()
    loss = total / max(count, 1.0) if count > 0 else 0.0
    return np.asarray(loss, dtype=np.float32)


# revision 4
# speedup vs baseline: 1.2562x; 1.2562x over previous
"""Trainium2 Bass kernel for ColumnConsistencyLoss (segment_reduce).

Problem: B=16, T=8192, C=128.
  probs = softmax(logits, -1)           # (N, C), N = B*T = 131072
  per column-id c (segment): n_c = #valid tokens, S_c = sum w*p, Q_c = sum w*p^2
  col_var_c = (sum_j Q_cj - sum_j S_cj^2 / n_safe_c) / (n_safe_c * C)
  loss = mean over columns with n_c > 1 of col_var_c

Sharding: data-parallel over tokens — each of the 8 cores processes
N/8 = 16384 tokens and produces partial per-segment accumulators
S (C x C) and Q (C x C).  The cross-core reduction of these tiny
accumulators plus the final scalar math happens on the host (exact
counts n_c are computed on host via bincount).

Device kernel per core (v2 — single streaming sweep, tokens on partitions):
  - host precomputes the one-hot segment matrix M (fp8, exact 0/1)
  - DMA logits in 4 chunks of (128p, 32j, 128c)   [token t = p*128 + j]
  - ScalarE: E = exp(L) -> bf16 (big-FD chunk)
  - DVE:     d = rowsum(E) via 2x halving adds (bf16 2x mode) + reduce
  - DVE:     r = 1/d ; a = w * r (tiny)
  - DVE:     rhs[:, :, 0, :] = E * broadcast(a)   (one chunk-level TT)
  - ScalarE: rhs[:, :, 1, :] = Square(rhs[:, :, 0, :])
  - PE:      psum[(c),(s,j')] += M_j^T @ rhs[:,j,:,:]  (F=256, fp32 accum)
The matmul contracts the 128 partitions (tokens):
  psum[c,0,:] = sum_t 1[seg=c] * (w*E/d)        = S_c
  psum[c,1,:] = sum_t 1[seg=c] * (w*E/d)^2      = Q_c   (w^2 = w).
"""

import numpy as np
import ml_dtypes

NCORES = 8
P = 128           # partitions
C = 128           # columns / segments
B, T = 16, 8192
N_TOK = B * T
TOK_PER_CORE = N_TOK // NCORES   # 16384
J_FULL = TOK_PER_CORE // P       # 128 free-columns (token tiles) per core
CHUNK_J = 32                     # token tiles per DMA/compute chunk

TRACE = False          # set True (e.g. from test.py) to capture NTFF profile
TRACE_TMPDIR = None    # where trace/NEFF artifacts land when TRACE is set
LAST_RESULT = None     # BassKernelResults of the last run (for profiling)

_NC_CACHE = {}


def build_nc(j_full=J_FULL, chunk_j=CHUNK_J):
    """Build + compile the Bass program (SPMD; same NEFF on all cores)."""
    from concourse import bacc, mybir
    import concourse.tile as tile

    f32 = mybir.dt.float32
    bf16 = mybir.dt.bfloat16
    fp8 = mybir.dt.float8e4
    Exp = mybir.ActivationFunctionType.Exp
    Square = mybir.ActivationFunctionType.Square
    Alu = mybir.AluOpType

    tok = j_full * P
    nchunk = j_full // chunk_j
    H = C // 2   # 64
    Q4 = C // 4  # 32

    nc = bacc.Bacc("TRN2", target_bir_lowering=False, debug=False,
                   enable_asserts=False)

    lg_d = nc.dram_tensor("logits", [tok, C], f32, kind="ExternalInput")
    m_d = nc.dram_tensor("m8", [tok, C], fp8, kind="ExternalInput")
    w_d = nc.dram_tensor("wf", [P, j_full], f32, kind="ExternalInput")
    sq_d = nc.dram_tensor("sq_out", [C, 2, C], f32, kind="ExternalOutput")

    with tile.TileContext(nc) as tc:
        with (
            tc.tile_pool(name="const", bufs=1) as constp,
            tc.tile_pool(name="big", bufs=3) as bigp,
            tc.tile_pool(name="small", bufs=3) as smallp,
            tc.tile_pool(name="psum", bufs=1, space="PSUM") as psump,
        ):
            w_t = constp.tile([P, j_full], f32)
            nc.sync.dma_start(w_t[:], w_d[:])

            psum_sq = psump.tile([C, 2, C], f32)

            # DRAM views: (p, j, c) with token t = p*j_full + j
            lg_ap = lg_d[:].rearrange("(p j) c -> p j c", j=j_full)
            m_ap = m_d[:].rearrange("(p j) c -> p j c", j=j_full)

            for k in range(nchunk):
                js = k * chunk_j
                L = bigp.tile([P, chunk_j, C], f32, tag="L")
                nc.sync.dma_start(L[:], lg_ap[:, js:js + chunk_j, :])
                M8 = bigp.tile([P, chunk_j, C], fp8, tag="M8")
                nc.sync.dma_start(M8[:], m_ap[:, js:js + chunk_j, :])

                E = bigp.tile([P, chunk_j, C], bf16, tag="E")
                nc.scalar.activation(E[:], L[:], Exp)

                # d = rowsum(E): two bf16 2x halving adds + 1x reduce
                h1 = bigp.tile([P, chunk_j, H], bf16, tag="h1")
                nc.vector.tensor_tensor(h1[:], E[:, :, 0:H], E[:, :, H:C],
                                        op=Alu.add)
                h2 = bigp.tile([P, chunk_j, Q4], bf16, tag="h2")
                nc.vector.tensor_tensor(h2[:], h1[:, :, 0:Q4], h1[:, :, Q4:H],
                                        op=Alu.add)
                d = smallp.tile([P, chunk_j], f32, tag="d")
                nc.vector.tensor_reduce(d[:], h2[:], axis=mybir.AxisListType.X,
                                        op=Alu.add)
                r = smallp.tile([P, chunk_j], f32, tag="r")
                nc.vector.reciprocal(r[:], d[:])
                a = smallp.tile([P, chunk_j], f32, tag="a")
                nc.vector.tensor_tensor(a[:], r[:], w_t[:, js:js + chunk_j],
                                        op=Alu.mult)

                rhs = bigp.tile([P, chunk_j, 2, C], bf16, tag="rhs")
                # normalized probs: one chunk-level TT with broadcast in1
                nc.vector.tensor_tensor(
                    rhs[:, :, 0, :], E[:],
                    a[:, :, None].to_broadcast([P, chunk_j, C]),
                    op=Alu.mult)
                # squared half for the whole chunk in one ACT instruction
                nc.scalar.activation(rhs[:, :, 1, :], rhs[:, :, 0, :], Square)

                for jj in range(chunk_j):
                    j = js + jj
                    nc.tensor.matmul(
                        psum_sq[:], M8[:, jj, :], rhs[:, jj, :, :],
                        start=(j == 0), stop=(j == j_full - 1))

            out_t = constp.tile([C, 2, C], f32)
            nc.scalar.copy(out_t[:], psum_sq[:])
            nc.sync.dma_start(sq_d[:], out_t[:])

    nc.compile()
    return nc


def _get_nc():
    key = (J_FULL, CHUNK_J)
    if key not in _NC_CACHE:
        _NC_CACHE[key] = build_nc(*key)
    return _NC_CACHE[key]


def kernel(column_logits, column_assignments, valid_mask):
    global LAST_RESULT
    from concourse.bass_utils import run_bass_kernel_spmd

    logits = np.asarray(column_logits, dtype=np.float32).reshape(N_TOK, C)
    seg = np.asarray(column_assignments).reshape(N_TOK).astype(np.int64)
    w = np.asarray(valid_mask).reshape(N_TOK).astype(bool)

    fp8np = ml_dtypes.float8_e4m3
    M8_full = np.zeros((N_TOK, C), dtype=fp8np)
    M8_full[np.arange(N_TOK), seg] = fp8np(1.0)

    in_maps = []
    for i in range(NCORES):
        sl = slice(i * TOK_PER_CORE, (i + 1) * TOK_PER_CORE)
        in_maps.append({
            "logits": np.ascontiguousarray(logits[sl]),
            "m8": np.ascontiguousarray(M8_full[sl]),
            "wf": np.ascontiguousarray(
                w[sl].reshape(P, J_FULL).astype(np.float32)),
        })

    nc = _get_nc()
    res = run_bass_kernel_spmd(nc, in_maps, list(range(NCORES)), trace=TRACE,
                               tmpdir=TRACE_TMPDIR)
    LAST_RESULT = res

    SQ = np.zeros((C, 2, C), np.float64)
    for rm in res.results:
        SQ += np.asarray(rm["sq_out"], dtype=np.float64)
    S = SQ[:, 0, :]
    Q = SQ[:, 1, :]

    n = np.bincount(seg[w], minlength=C).astype(np.float64)
    n_safe = np.maximum(n, 1.0)
    ssd_sum = Q.sum(axis=1) - (S * S).sum(axis=1) / n_safe
    col_var = ssd_sum / (n_safe * C)
    has_multi = n > 1.0
    count = has_multi.sum()
    total = np.where(has_multi, col_var, 0.0).sum()
    loss = total / max(count, 1.0) if count > 0 else 0.0
    return np.asarray(loss, dtype=np.float32)


# revision 8
# speedup vs baseline: 1.3500x; 1.0747x over previous
"""Trainium2 Bass kernel for ColumnConsistencyLoss (segment_reduce).

Problem: B=16, T=8192, C=128.
  probs = softmax(logits, -1)           # (N, C), N = B*T = 131072
  per column-id c (segment): n_c = #valid tokens, S_c = sum w*p, Q_c = sum w*p^2
  col_var_c = (sum_j Q_cj - sum_j S_cj^2 / n_safe_c) / (n_safe_c * C)
  loss = mean over columns with n_c > 1 of col_var_c

Sharding: data-parallel over tokens — each of the 8 cores processes
N/8 = 16384 tokens and produces partial per-segment accumulators
S (C x C) and Q (C x C).  The cross-core reduction of these tiny
accumulators plus the final scalar math happens on the host (exact
counts n_c are computed on host via bincount).

Device kernel per core (v3 — single streaming sweep, tokens on partitions):
  - host precomputes M = onehot(seg) * w  (fp8, exact 0/1 values)
  - DMA logits in non-uniform chunks [8,16,32,32,32,8] tiles (small first
    chunk fills the pipeline early; small last chunk shortens the tail)
  - ScalarE: E = exp(L) -> bf16 (big-FD chunk)
  - DVE:     d = rowsum(E) via 2x halving adds (bf16 2x mode) + reduce
  - DVE:     r ~= 1/d (reciprocal_approx_fast, ~51 ULP)
  - DVE:     rhs[:, :, 0, :] = E * broadcast(r)   (one chunk-level TT)
  - ScalarE: rhs[:, :, 1, :] = Square(rhs[:, :, 0, :])  (2 half-chunk ACTs)
  - PE:      psum[(c),(s,j')] += M_j^T @ rhs[:,j,:,:]  (F=256, fp32 accum)
The matmul contracts the 128 partitions (tokens); w rides in M:
  psum[c,0,:] = sum_t w*1[seg=c] * (E/d)        = S_c
  psum[c,1,:] = sum_t w*1[seg=c] * (E/d)^2      = Q_c   (w^2 = w).
"""

import numpy as np
import ml_dtypes

NCORES = 8
P = 128           # partitions
C = 128           # columns / segments
B, T = 16, 8192
N_TOK = B * T
TOK_PER_CORE = N_TOK // NCORES   # 16384
J_FULL = TOK_PER_CORE // P       # 128 free-columns (token tiles) per core
CHUNKS = (8, 16, 32, 32, 32, 8)  # token tiles per DMA/compute chunk

TRACE = False          # set True (e.g. from test.py) to capture NTFF profile
TRACE_TMPDIR = None    # where trace/NEFF artifacts land when TRACE is set
LAST_RESULT = None     # BassKernelResults of the last run (for profiling)

_NC_CACHE = {}


def build_nc(chunks=CHUNKS):
    """Build + compile the Bass program (SPMD; same NEFF on all cores)."""
    from concourse import bacc, mybir
    import concourse.tile as tile

    f32 = mybir.dt.float32
    bf16 = mybir.dt.bfloat16
    fp8 = mybir.dt.float8e4
    Exp = mybir.ActivationFunctionType.Exp
    Square = mybir.ActivationFunctionType.Square
    Alu = mybir.AluOpType

    j_full = sum(chunks)
    tok = j_full * P
    H = C // 2   # 64
    Q4 = C // 4  # 32

    nc = bacc.Bacc("TRN2", target_bir_lowering=False, debug=False,
                   enable_asserts=False)

    lg_d = nc.dram_tensor("logits", [tok, C], f32, kind="ExternalInput")
    m_d = nc.dram_tensor("m8", [tok, C], fp8, kind="ExternalInput")
    sq_d = nc.dram_tensor("sq_out", [C, 2, C], f32, kind="ExternalOutput")

    with tile.TileContext(nc) as tc:
        with (
            tc.tile_pool(name="const", bufs=1) as constp,
            tc.tile_pool(name="big", bufs=3) as bigp,
            tc.tile_pool(name="small", bufs=4) as smallp,
            tc.tile_pool(name="psum", bufs=1, space="PSUM") as psump,
        ):
            psum_sq = psump.tile([C, 2, C], f32)

            # DRAM views: (p, j, c) with token t = p*j_full + j
            lg_ap = lg_d[:].rearrange("(p j) c -> p j c", j=j_full)
            m_ap = m_d[:].rearrange("(p j) c -> p j c", j=j_full)

            js = 0
            for k, cj in enumerate(chunks):
                L = bigp.tile([P, cj, C], f32, tag="L")
                nc.sync.dma_start(L[:], lg_ap[:, js:js + cj, :])
                M8 = bigp.tile([P, cj, C], fp8, tag="M8")
                nc.sync.dma_start(M8[:], m_ap[:, js:js + cj, :])

                E = bigp.tile([P, cj, C], bf16, tag="E")
                nc.scalar.activation(E[:], L[:], Exp)

                # d = rowsum(E): two bf16 2x halving adds + 1x reduce
                h1 = bigp.tile([P, cj, H], bf16, tag="h1")
                nc.vector.tensor_tensor(h1[:], E[:, :, 0:H], E[:, :, H:C],
                                        op=Alu.add)
                h2 = bigp.tile([P, cj, Q4], bf16, tag="h2")
                nc.vector.tensor_tensor(h2[:], h1[:, :, 0:Q4], h1[:, :, Q4:H],
                                        op=Alu.add)
                d = smallp.tile([P, cj], f32, tag="d")
                nc.vector.tensor_reduce(d[:], h2[:], axis=mybir.AxisListType.X,
                                        op=Alu.add)
                r = smallp.tile([P, cj], f32, tag="r")
                nc.vector.reciprocal_approx_fast(r[:], d[:])

                rhs = bigp.tile([P, cj, 2, C], bf16, tag="rhs")
                # normalized probs: one chunk-level TT with broadcast in1
                nc.vector.tensor_tensor(
                    rhs[:, :, 0, :], E[:],
                    r[:, :, None].to_broadcast([P, cj, C]),
                    op=Alu.mult)
                # squared half in two ACTs so matmuls can start earlier
                hh = max(cj // 2, 1)
                nc.scalar.activation(rhs[:, 0:hh, 1, :], rhs[:, 0:hh, 0, :],
                                     Square)
                if hh < cj:
                    nc.scalar.activation(rhs[:, hh:cj, 1, :],
                                         rhs[:, hh:cj, 0, :], Square)

                for jj in range(cj):
                    j = js + jj
                    nc.tensor.matmul(
                        psum_sq[:], M8[:, jj, :], rhs[:, jj, :, :],
                        start=(j == 0), stop=(j == j_full - 1))
                js += cj

            out_t = constp.tile([C, 2, C], f32)
            nc.scalar.copy(out_t[:], psum_sq[:])
            nc.sync.dma_start(sq_d[:], out_t[:])

    nc.compile()
    return nc


def _get_nc():
    key = CHUNKS
    if key not in _NC_CACHE:
        _NC_CACHE[key] = build_nc(key)
    return _NC_CACHE[key]


def kernel(column_logits, column_assignments, valid_mask):
    global LAST_RESULT
    from concourse.bass_utils import run_bass_kernel_spmd

    logits = np.asarray(column_logits, dtype=np.float32).reshape(N_TOK, C)
    seg = np.asarray(column_assignments).reshape(N_TOK).astype(np.int64)
    w = np.asarray(valid_mask).reshape(N_TOK).astype(bool)

    fp8np = ml_dtypes.float8_e4m3
    M8_full = np.zeros((N_TOK, C), dtype=fp8np)
    M8_full[np.arange(N_TOK)[w], seg[w]] = fp8np(1.0)   # w folded into M

    in_maps = []
    for i in range(NCORES):
        sl = slice(i * TOK_PER_CORE, (i + 1) * TOK_PER_CORE)
        in_maps.append({
            "logits": np.ascontiguousarray(logits[sl]),
            "m8": np.ascontiguousarray(M8_full[sl]),
        })

    nc = _get_nc()
    res = run_bass_kernel_spmd(nc, in_maps, list(range(NCORES)), trace=TRACE,
                               tmpdir=TRACE_TMPDIR)
    LAST_RESULT = res

    SQ = np.zeros((C, 2, C), np.float64)
    for rm in res.results:
        SQ += np.asarray(rm["sq_out"], dtype=np.float64)
    S = SQ[:, 0, :]
    Q = SQ[:, 1, :]

    n = np.bincount(seg[w], minlength=C).astype(np.float64)
    n_safe = np.maximum(n, 1.0)
    ssd_sum = Q.sum(axis=1) - (S * S).sum(axis=1) / n_safe
    col_var = ssd_sum / (n_safe * C)
    has_multi = n > 1.0
    count = has_multi.sum()
    total = np.where(has_multi, col_var, 0.0).sum()
    loss = total / max(count, 1.0) if count > 0 else 0.0
    return np.asarray(loss, dtype=np.float32)


# revision 11
# speedup vs baseline: 1.5417x; 1.1420x over previous
"""Trainium2 Bass kernel for ColumnConsistencyLoss (segment_reduce).

Problem: B=16, T=8192, C=128.
  probs = softmax(logits, -1)           # (N, C), N = B*T = 131072
  per column-id c (segment): n_c = #valid tokens, S_c = sum w*p, Q_c = sum w*p^2
  col_var_c = (sum_j Q_cj - sum_j S_cj^2 / n_safe_c) / (n_safe_c * C)
  loss = mean over columns with n_c > 1 of col_var_c

Sharding: data-parallel over tokens — each of the 8 cores processes
N/8 = 16384 tokens and produces partial per-segment accumulators
S (C x C) and Q (C x C).  The cross-core reduction of these tiny
accumulators plus the final scalar math happens on the host (exact
counts n_c are computed on host via bincount).

Device kernel per core (v3 — single streaming sweep, tokens on partitions):
  - host precomputes M = onehot(seg) * w  (fp8, exact 0/1 values)
  - DMA logits in non-uniform chunks [8,16,32,32,32,8] tiles (small first
    chunk fills the pipeline early; small last chunk shortens the tail)
  - ScalarE: E = exp(L) -> bf16 (big-FD chunk)
  - DVE:     d = rowsum(E) via 2x halving adds (bf16 2x mode) + reduce
  - DVE:     r ~= 1/d (reciprocal_approx_fast, ~51 ULP)
  - DVE:     rhs[:, :, 0, :] = E * broadcast(r)   (one chunk-level TT)
  - ScalarE: rhs[:, :, 1, :] = Square(rhs[:, :, 0, :])  (2 half-chunk ACTs)
  - PE:      psum[(c),(s,j')] += M_j^T @ rhs[:,j,:,:]  (F=256, fp32 accum)
The matmul contracts the 128 partitions (tokens); w rides in M:
  psum[c,0,:] = sum_t w*1[seg=c] * (E/d)        = S_c
  psum[c,1,:] = sum_t w*1[seg=c] * (E/d)^2      = Q_c   (w^2 = w).
"""

import numpy as np
import ml_dtypes

NCORES = 8
P = 128           # partitions
C = 128           # columns / segments
B, T = 16, 8192
N_TOK = B * T
TOK_PER_CORE = N_TOK // NCORES   # 16384
J_FULL = TOK_PER_CORE // P       # 128 free-columns (token tiles) per core
CHUNKS = (4, 12, 24, 28, 28, 24, 8)  # token tiles per DMA/compute chunk

TRACE = False          # set True (e.g. from test.py) to capture NTFF profile
TRACE_TMPDIR = None    # where trace/NEFF artifacts land when TRACE is set
LAST_RESULT = None     # BassKernelResults of the last run (for profiling)

_NC_CACHE = {}


def build_nc(chunks=CHUNKS):
    """Build + compile the Bass program (SPMD; same NEFF on all cores)."""
    from concourse import bacc, mybir
    import concourse.tile as tile

    f32 = mybir.dt.float32
    bf16 = mybir.dt.bfloat16
    fp8 = mybir.dt.float8e4
    Exp = mybir.ActivationFunctionType.Exp
    Square = mybir.ActivationFunctionType.Square
    Alu = mybir.AluOpType

    j_full = sum(chunks)
    tok = j_full * P
    H = C // 2   # 64
    Q4 = C // 4  # 32

    nc = bacc.Bacc("TRN2", target_bir_lowering=False, debug=False,
                   enable_asserts=False)

    lg_d = nc.dram_tensor("logits", [tok, C], f32, kind="ExternalInput")
    m_d = nc.dram_tensor("m8", [tok, C], fp8, kind="ExternalInput")
    sq_d = nc.dram_tensor("sq_out", [C, 2, C], f32, kind="ExternalOutput")

    with tile.TileContext(nc) as tc:
        with (
            tc.tile_pool(name="const", bufs=1) as constp,
            tc.tile_pool(name="ld", bufs=4) as ldp,
            tc.tile_pool(name="big", bufs=3) as bigp,
            tc.tile_pool(name="small", bufs=4) as smallp,
            tc.tile_pool(name="psum", bufs=1, space="PSUM") as psump,
        ):
            psum_sq = psump.tile([C, 2, C], f32)

            # DRAM views: (p, j, c) with token t = p*j_full + j
            lg_ap = lg_d[:].rearrange("(p j) c -> p j c", j=j_full)
            m_ap = m_d[:].rearrange("(p j) c -> p j c", j=j_full)

            js = 0
            for k, cj in enumerate(chunks):
                L = ldp.tile([P, cj, C], f32, tag="L")
                nc.sync.dma_start(L[:], lg_ap[:, js:js + cj, :])
                M8 = ldp.tile([P, cj, C], fp8, tag="M8")
                # scalar-issued DMA rides the second HWDGE ring, so the
                # one-hot load doesn't queue behind the logits stream
                nc.scalar.dma_start(M8[:], m_ap[:, js:js + cj, :])

                E = bigp.tile([P, cj, C], bf16, tag="E")
                nc.scalar.activation(E[:], L[:], Exp)

                # d = rowsum(E): two bf16 2x halving adds + 1x reduce
                h1 = bigp.tile([P, cj, H], bf16, tag="h1")
                nc.vector.tensor_tensor(h1[:], E[:, :, 0:H], E[:, :, H:C],
                                        op=Alu.add)
                h2 = bigp.tile([P, cj, Q4], bf16, tag="h2")
                nc.vector.tensor_tensor(h2[:], h1[:, :, 0:Q4], h1[:, :, Q4:H],
                                        op=Alu.add)
                d = smallp.tile([P, cj], f32, tag="d")
                nc.vector.tensor_reduce(d[:], h2[:], axis=mybir.AxisListType.X,
                                        op=Alu.add)
                r = smallp.tile([P, cj], f32, tag="r")
                nc.vector.reciprocal_approx_fast(r[:], d[:])

                rhs = bigp.tile([P, cj, 2, C], bf16, tag="rhs")
                # normalized probs: one chunk-level TT with broadcast in1
                nc.vector.tensor_tensor(
                    rhs[:, :, 0, :], E[:],
                    r[:, :, None].to_broadcast([P, cj, C]),
                    op=Alu.mult)
                # squared half in two ACTs so matmuls can start earlier
                hh = max(cj // 2, 1)
                nc.scalar.activation(rhs[:, 0:hh, 1, :], rhs[:, 0:hh, 0, :],
                                     Square)
                if hh < cj:
                    nc.scalar.activation(rhs[:, hh:cj, 1, :],
                                         rhs[:, hh:cj, 0, :], Square)

                for jj in range(cj):
                    j = js + jj
                    nc.tensor.matmul(
                        psum_sq[:], M8[:, jj, :], rhs[:, jj, :, :],
                        start=(j == 0), stop=(j == j_full - 1))
                js += cj

            out_t = constp.tile([C, 2, C], f32)
            nc.scalar.copy(out_t[:], psum_sq[:])
            nc.sync.dma_start(sq_d[:], out_t[:])

    nc.compile()
    return nc


def _get_nc():
    key = CHUNKS
    if key not in _NC_CACHE:
        _NC_CACHE[key] = build_nc(key)
    return _NC_CACHE[key]


def kernel(column_logits, column_assignments, valid_mask):
    global LAST_RESULT
    from concourse.bass_utils import run_bass_kernel_spmd

    logits = np.asarray(column_logits, dtype=np.float32).reshape(N_TOK, C)
    seg = np.asarray(column_assignments).reshape(N_TOK).astype(np.int64)
    w = np.asarray(valid_mask).reshape(N_TOK).astype(bool)

    fp8np = ml_dtypes.float8_e4m3
    M8_full = np.zeros((N_TOK, C), dtype=fp8np)
    M8_full[np.arange(N_TOK)[w], seg[w]] = fp8np(1.0)   # w folded into M

    in_maps = []
    for i in range(NCORES):
        sl = slice(i * TOK_PER_CORE, (i + 1) * TOK_PER_CORE)
        in_maps.append({
            "logits": np.ascontiguousarray(logits[sl]),
            "m8": np.ascontiguousarray(M8_full[sl]),
        })

    nc = _get_nc()
    res = run_bass_kernel_spmd(nc, in_maps, list(range(NCORES)), trace=TRACE,
                               tmpdir=TRACE_TMPDIR)
    LAST_RESULT = res

    SQ = np.zeros((C, 2, C), np.float64)
    for rm in res.results:
        SQ += np.asarray(rm["sq_out"], dtype=np.float64)
    S = SQ[:, 0, :]
    Q = SQ[:, 1, :]

    n = np.bincount(seg[w], minlength=C).astype(np.float64)
    n_safe = np.maximum(n, 1.0)
    ssd_sum = Q.sum(axis=1) - (S * S).sum(axis=1) / n_safe
    col_var = ssd_sum / (n_safe * C)
    has_multi = n > 1.0
    count = has_multi.sum()
    total = np.where(has_multi, col_var, 0.0).sum()
    loss = total / max(count, 1.0) if count > 0 else 0.0
    return np.asarray(loss, dtype=np.float32)
